# revision 1
# baseline (speedup 1.0000x reference)
"""DecayMaskedMultiHeadAttention on 8 trn2 NeuronCores (Bass/Tile SPMD).

Model: B=4, N=1024, DIM=1024, 16 heads x head_dim 64.
  q/k/v = x @ W.T + b ; scores = (q_h k_h^T)/8 * decaymask_h ;
  out = softmax(scores) v_h ; y = concat_h(out) @ wo.T + bo

Sharding (8 cores): 4 head-groups x 2 batch-groups.
  core c: head group g = c // 2 (heads 4g..4g+3), batch group p = c % 2
  (batches 2p, 2p+1). Each core computes a partial y for its 2 batches
  (its 4 heads' contribution through the out-projection); host sums the
  4 partials per batch group and adds the closed-form bias terms
  (bo + bv @ wo.T; attention rows sum to 1 so bv passes through).

On-core layout (all matmuls in float32r, fp32 PSUM accumulate):
  - host pre-transposes x -> xT [D, N] and decaymask -> maskT [k, q],
    plus weight slices, so no on-chip transposes are needed anywhere.
  - qT/kT [head_dim, tok] per 2-head stack; v natural [tok, dim] with a
    ones column appended per head (gives the softmax denominator as an
    extra output row of the attn@v matmul).
  - scores are computed transposed (scoresT [k, q] = kT.T @ qT), decay
    mask applied on DVE, exp on ACT; exp needs no max-subtraction
    here (scores are O(1) * mask in [0,1)).
  - attn@v: out_hT [65, q] = [v_h | 1].T @ expT, accumulated over k.
    Row 64 is the denominator; reciprocal + PE broadcast + DVE multiply
    normalizes rows 0..63 while evacuating PSUM.
  - out-projection: lhsT is exactly the stacked normalized out_hT, so
    partial y [tok, D] = lhsT.T @ woT accumulates over the 2 stacks.
  - 1/sqrt(head_dim) = 1/8 is folded into wq/bq on the host (exact).
"""

import numpy as np
import ml_dtypes

DIM = 1024
H = 16
HD = 64
B = 4
N = 1024
NCORES = 8
HPC = 4            # heads per core
BPC = 2            # batches per core
NSTACK = 2         # 2-head stacks per core
VBLK = HD + 1      # v block width per head (ones column appended)
VROW = HPC * VBLK  # v columns per 128-token chunk


_PROGRAM = None  # (nc, input_names) cache
LAST_RESULTS = None  # BassKernelResults from the most recent run (for test.py)


def _build_program(reps=1):
    import concourse.mybir as mybir
    import concourse.tile as tile
    from concourse import bacc

    f32 = mybir.dt.float32
    f32r = mybir.dt.float32r
    f16 = mybir.dt.float16
    AF = mybir.ActivationFunctionType

    nc = bacc.Bacc(
        "TRN2",
        target_bir_lowering=False,
        debug=False,
        num_devices=NCORES,
    )

    xT = nc.dram_tensor("xT", [BPC, DIM, N], f16, kind="ExternalInput").ap()
    maskT = nc.dram_tensor("maskT", [HPC, N, N], f16, kind="ExternalInput").ap()
    wqT = nc.dram_tensor("wqT", [DIM, HPC * HD], f16, kind="ExternalInput").ap()
    wkT = nc.dram_tensor("wkT", [DIM, HPC * HD], f16, kind="ExternalInput").ap()
    wvT = nc.dram_tensor("wvT", [DIM, HPC * HD], f16, kind="ExternalInput").ap()
    woT = nc.dram_tensor("woT", [HPC * HD, DIM], f16, kind="ExternalInput").ap()
    bqd = nc.dram_tensor("bq", [NSTACK, 128, 1], f32, kind="ExternalInput").ap()
    bkd = nc.dram_tensor("bk", [NSTACK, 128, 1], f32, kind="ExternalInput").ap()
    outp = nc.dram_tensor("outp", [BPC, N, DIM], f32, kind="ExternalOutput").ap()

    KC = DIM // 128  # 8 contraction chunks over D
    TC = N // 128    # 8 token chunks
    QH = N // 512    # 2 q halves (fp32 moving-operand limit is 512)

    with tile.TileContext(nc) as tc:
        with (
            tc.tile_pool(name="w", bufs=1) as wpool,
            tc.tile_pool(name="persist", bufs=1) as persist,
            tc.tile_pool(name="xt", bufs=16) as xpool,
            tc.tile_pool(name="maskp", bufs=16) as maskp,
            tc.tile_pool(name="expm", bufs=3) as expm_p,
            tc.tile_pool(name="expo", bufs=4) as expo_p,
            tc.tile_pool(name="ev", bufs=3) as ev_p,
            tc.tile_pool(name="small", bufs=2) as small_p,
            tc.tile_pool(name="psA", bufs=2, space="PSUM") as psA,
            tc.tile_pool(name="psS", bufs=2, space="PSUM") as psS,
            tc.tile_pool(name="psO", bufs=2, space="PSUM") as psO,
        ):
            # --- weights; actual DMAs are interleaved with x0 below ---
            wq_t = [wpool.tile([128, HPC * HD], f16, tag=f"wq{kc}", name=f"wq{kc}") for kc in range(KC)]
            wk_t = [wpool.tile([128, HPC * HD], f16, tag=f"wk{kc}", name=f"wk{kc}") for kc in range(KC)]
            wv_t = [wpool.tile([128, HPC * HD], f16, tag=f"wv{kc}", name=f"wv{kc}") for kc in range(KC)]
            bq_t = [wpool.tile([128, 1], f32, tag=f"bq{s}", name=f"bq{s}") for s in range(NSTACK)]
            bk_t = [wpool.tile([128, 1], f32, tag=f"bk{s}", name=f"bk{s}") for s in range(NSTACK)]

            # persistent activations
            qT = {}
            kT = {}
            ao = {}
            for b in range(BPC):
                for s in range(NSTACK):
                    qT[(b, s)] = persist.tile([128, N], f16, tag=f"qT{b}{s}", name=f"qT{b}{s}")
                    kT[(b, s)] = persist.tile([128, N], f16, tag=f"kT{b}{s}", name=f"kT{b}{s}")
                    ao[(b, s)] = persist.tile([128, N], f16, tag=f"ao{b}{s}", name=f"ao{b}{s}")
            vt = {b: persist.tile([128, TC * VROW], f16, tag=f"v{b}", name=f"v{b}") for b in range(BPC)}
            for b in range(BPC):
                # whole-tile fill; projection evacs overwrite the data
                # columns, leaving the per-head ones column = 1.0
                nc.vector.memset(vt[b][:], 1.0)

            xts = {}

            def load_x(b):
                for kc in range(KC):
                    t = xpool.tile([128, N], f16, tag="xts", name=f"xts{b}_{kc}")
                    nc.sync.dma_start(t[:], xT[b, kc * 128:(kc + 1) * 128, :])
                    xts[(b, kc)] = t

            def load_startup():
                """x0 + weights interleaved per kc so the first projection
                matmul can start after ~1 MB of DMA."""
                for kc in range(KC):
                    t = xpool.tile([128, N], f16, tag="xts", name=f"xts0_{kc}")
                    nc.sync.dma_start(t[:], xT[0, kc * 128:(kc + 1) * 128, :])
                    xts[(0, kc)] = t
                    nc.sync.dma_start(wq_t[kc][:], wqT[kc * 128:(kc + 1) * 128, :])
                    nc.sync.dma_start(wk_t[kc][:], wkT[kc * 128:(kc + 1) * 128, :])
                    nc.sync.dma_start(wv_t[kc][:], wvT[kc * 128:(kc + 1) * 128, :])
                    if kc == 0:
                        for s in range(NSTACK):
                            nc.sync.dma_start(bq_t[s][:], bqd[s])
                            nc.sync.dma_start(bk_t[s][:], bkd[s])
                for s in range(NSTACK):
                    nc.sync.dma_start(wo_t[s][:], woT[s * 128:(s + 1) * 128, :])

            def qk_group(b, wt, bt, dst, s, qh):
                ps = psA.tile([128, 512], f32, tag="big",
                              name=f"pj{b}{s}{qh}{'q' if wt is wq_t else 'k'}")
                for kc in range(KC):
                    nc.tensor.matmul(
                        ps[:],
                        lhsT=wt[kc][:, s * 128:(s + 1) * 128],
                        rhs=xts[(b, kc)][:, qh * 512:(qh + 1) * 512],
                        start=(kc == 0),
                        stop=(kc == KC - 1),
                    )
                nc.scalar.activation(
                    dst[(b, s)][:, qh * 512:(qh + 1) * 512],
                    ps[:],
                    AF.Identity,
                    bias=bt[s][:],
                    scale=1.0,
                )

            def v_group(b, tci):
                ps = psA.tile([128, 512], f32, tag="big", name=f"pjv{b}{tci}")
                for kc in range(KC):
                    nc.tensor.matmul(
                        ps[:, 0:HPC * HD],
                        lhsT=xts[(b, kc)][:, tci * 128:(tci + 1) * 128],
                        rhs=wv_t[kc][:],
                        start=(kc == 0),
                        stop=(kc == KC - 1),
                    )
                dst = vt[b][:, tci * VROW:(tci + 1) * VROW].rearrange(
                    "p (h e) -> p h e", e=VBLK
                )[:, :, 0:HD]
                nc.scalar.activation(
                    dst, ps[:, 0:HPC * HD].rearrange("p (h e) -> p h e", e=HD),
                    AF.Copy,
                )

            def qk_groups(b, s):
                out = []
                for wt, bt, dst in ((wq_t, bq_t, qT), (wk_t, bk_t, kT)):
                    for qh in range(QH):
                        out.append(lambda b=b, wt=wt, bt=bt, dst=dst, s=s, qh=qh:
                                   qk_group(b, wt, bt, dst, s, qh))
                return out

            mask_t = {}

            def preload_mask(h):
                for kc in range(TC):
                    mt = maskp.tile([128, N], f16, tag="mask", name=f"mask{h}_{kc}")
                    nc.gpsimd.dma_start(mt[:], maskT[h, kc * 128:(kc + 1) * 128, :])
                    mask_t[(h, kc)] = mt

            def attn(h, b, fillers=None, per_slot=1):
                """Attention for head h (local), batch b. b=0 loads mask tiles.
                Pops up to per_slot PE filler groups per kc iteration."""
                s, hh = h // 2, h % 2
                op = {}
                for qh in range(QH):
                    op[qh] = psO.tile([VBLK, 512], f32, tag=f"ov{qh}", name=f"ov{h}{b}{qh}")
                if b == 1 and h + 2 < HPC:
                    preload_mask(h + 2)
                for kc in range(TC):
                    if fillers:
                        for _ in range(per_slot):
                            if fillers:
                                fillers.pop(0)()
                    mt = mask_t[(h, kc)]
                    em = expm_p.tile([128, N], f32, tag="expm", name=f"expm{h}{kc}{b}")
                    for qh in range(QH):
                        sc = psS.tile([128, 512], f32, tag="sc", name=f"sc{h}{kc}{b}{qh}")
                        nc.tensor.matmul(
                            sc[:],
                            lhsT=kT[(b, s)][hh * HD:(hh + 1) * HD,
                                            kc * 128:(kc + 1) * 128],
                            rhs=qT[(b, s)][hh * HD:(hh + 1) * HD,
                                           qh * 512:(qh + 1) * 512],
                            start=True,
                            stop=True,
                        )
                        nc.vector.tensor_mul(
                            em[:, qh * 512:(qh + 1) * 512],
                            sc[:],
                            mt[:, qh * 512:(qh + 1) * 512],
                        )
                    eo = expo_p.tile([128, N], f16, tag="expo", name=f"expo{h}{kc}{b}")
                    nc.scalar.activation(eo[:], em[:], AF.Exp)
                    vblk = vt[b][:, kc * VROW + h * VBLK:kc * VROW + (h + 1) * VBLK]
                    for qh in range(QH):
                        nc.tensor.matmul(
                            op[qh][:],
                            lhsT=vblk,
                            rhs=eo[:, qh * 512:(qh + 1) * 512],
                            start=(kc == 0),
                            stop=(kc == TC - 1),
                        )
                for qh in range(QH):
                    rc = small_p.tile([1, 512], f32, tag="rec", name=f"rec{h}{b}{qh}")
                    nc.vector.reciprocal(rc[:], op[qh][HD:VBLK, :])
                    bcs = small_p.tile([HD, 512], f32, tag="bcs", name=f"bcs{h}{b}{qh}")
                    nc.gpsimd.partition_broadcast(bcs[:], rc[:])
                    nc.vector.tensor_mul(
                        ao[(b, s)][hh * HD:(hh + 1) * HD, qh * 512:(qh + 1) * 512],
                        op[qh][0:HD, :],
                        bcs[:],
                    )

            def outproj_group(b, tci, dh):
                        po = psA.tile([128, 512], f32, tag="big", name=f"po{b}{tci}{dh}")
                        for s in range(NSTACK):
                            nc.tensor.matmul(
                                po[:],
                                lhsT=ao[(b, s)][:, tci * 128:(tci + 1) * 128],
                                rhs=wo_t[s][:, dh * 512:(dh + 1) * 512],
                                start=(s == 0),
                                stop=(s == NSTACK - 1),
                            )
                        ot = ev_p.tile([128, 512], f32, tag="ot", name=f"ot{b}{tci}{dh}")
                        if tci % 2 == 0:
                            nc.vector.tensor_copy(ot[:], po[:])
                        else:
                            nc.scalar.copy(ot[:], po[:])
                        nc.sync.dma_start(
                            outp[b, tci * 128:(tci + 1) * 128,
                                 dh * 512:(dh + 1) * 512],
                            ot[:],
                        )

            wo_t = [wpool.tile([128, DIM], f16, tag=f"wo{s}", name=f"wo{s}") for s in range(NSTACK)]

            # Software-pipelined emission. Projections for stack s1 of batch 0
            # plus all of batch 1's projections ride as PE "fillers" inside
            # the attention kc loops; out-projection of batch 0 hides under
            # the final attention phase of batch 1. reps>1 repeats the whole
            # computation in-NEFF (timing amplification only).
            for _rep in range(reps):
                mask_t.clear()
                xts.clear()
                preload_mask(0)
                preload_mask(1)
                load_startup()
                for g in qk_groups(0, 0):
                    g()
                for tci in range(TC):
                    v_group(0, tci)
                load_x(1)
                fillers = (qk_groups(0, 1) + qk_groups(1, 0)
                           + [lambda tci=tci: v_group(1, tci) for tci in range(TC)]
                           + qk_groups(1, 1))
                attn(0, 0, fillers, per_slot=2)
                attn(0, 1, fillers, per_slot=1)
                attn(1, 0, fillers, per_slot=1)
                attn(1, 1, fillers, per_slot=1)
                attn(2, 0)
                attn(2, 1)
                attn(3, 0)
                fillers2 = [lambda b=0, tci=tci, dh=dh: outproj_group(0, tci, dh)
                            for tci in range(TC) for dh in range(QH)]
                attn(3, 1, fillers2, per_slot=2)
                for f in fillers2:
                    f()
                for tci in range(TC):
                    for dh in range(QH):
                        outproj_group(1, tci, dh)

    nc.compile()
    return nc


def _get_program():
    global _PROGRAM
    if _PROGRAM is None:
        _PROGRAM = _build_program()
    return _PROGRAM


def kernel(x, decaymask, wq, bq, wk, bk, wv, bv, wo, bo):
    from concourse.bass_utils import run_bass_kernel_spmd

    global LAST_RESULTS

    x = np.ascontiguousarray(np.asarray(x, dtype=np.float32))
    decaymask = np.ascontiguousarray(np.asarray(decaymask, dtype=np.float32))
    wq = np.asarray(wq, dtype=np.float32)
    bq = np.asarray(bq, dtype=np.float32)
    wk = np.asarray(wk, dtype=np.float32)
    bk = np.asarray(bk, dtype=np.float32)
    wv = np.asarray(wv, dtype=np.float32)
    bv = np.asarray(bv, dtype=np.float32)
    wo = np.asarray(wo, dtype=np.float32)
    bo = np.asarray(bo, dtype=np.float32)

    nc = _get_program()

    in_maps = []
    for c in range(NCORES):
        g, p = c // 2, c % 2
        rows = slice(g * HPC * HD, (g + 1) * HPC * HD)
        xT_c = np.ascontiguousarray(
            x[p * BPC:(p + 1) * BPC].transpose(0, 2, 1)
        ).astype(np.float16)  # [BPC, D, N]
        maskT_c = np.ascontiguousarray(
            decaymask[g * HPC:(g + 1) * HPC].transpose(0, 2, 1)
        ).astype(np.float16)  # [HPC, k, q]
        # fold 1/sqrt(HD) = 1/8 (exact) into wq/bq
        wqT_c = (np.ascontiguousarray(wq[rows, :].T) * np.float32(0.125)).astype(np.float16)
        wkT_c = np.ascontiguousarray(wk[rows, :].T).astype(np.float16)
        wvT_c = np.ascontiguousarray(wv[rows, :].T).astype(np.float16)
        woT_c = np.ascontiguousarray(wo[:, rows].T).astype(np.float16)
        bq_c = (bq[rows] * np.float32(0.125)).reshape(NSTACK, 128, 1)
        bk_c = bk[rows].reshape(NSTACK, 128, 1).copy()
        in_maps.append({
            "xT": xT_c,
            "maskT": maskT_c,
            "wqT": wqT_c,
            "wkT": wkT_c,
            "wvT": wvT_c,
            "woT": woT_c,
            "bq": np.ascontiguousarray(bq_c),
            "bk": bk_c,
        })

    res = run_bass_kernel_spmd(nc, in_maps, list(range(NCORES)))
    LAST_RESULTS = res

    out = np.zeros((B, N, DIM), dtype=np.float32)
    for c in range(NCORES):
        g, p = c // 2, c % 2
        out[p * BPC:(p + 1) * BPC] += res.results[c]["outp"]
    out += (bo + bv @ wo.T)[None, None, :]
    return out



# revision 5
# speedup vs baseline: 1.2053x; 1.2053x over previous
"""DecayMaskedMultiHeadAttention on 8 trn2 NeuronCores (Bass/Tile SPMD).

Model: B=4, N=1024, DIM=1024, 16 heads x head_dim 64.
  q/k/v = x @ W.T + b ; scores = (q_h k_h^T)/8 * decaymask_h ;
  out = softmax(scores) v_h ; y = concat_h(out) @ wo.T + bo

Sharding (8 cores): 4 head-groups x 2 batch-groups.
  core c: head group g = c // 2 (heads 4g..4g+3), batch group p = c % 2
  (batches 2p, 2p+1). Each core computes a partial y (f16) for its 2
  batches; host sums the 4 partials per batch group and adds the
  closed-form bias terms (bo + bv @ wo.T; attention rows sum to 1 so bv
  passes through).

Schedule notes (TRN2: Pool/GPSIMD cannot touch PSUM, so all PSUM-side
element-wise work lives on DVE + ACT):
  - Attention runs as TWO concurrent (head, batch) streams interleaved
    kc-by-kc with the attn@v matmuls software-pipelined one kc behind
    the scores, so the PE never waits on a single
    score->mask-mul->exp->attn@v chain (PE duty keeps the clock p-state
    at 2.4 GHz).
  - v tiles carry 64 ones-columns per head: the attn@v matmul emits the
    softmax denominator already replicated on partitions 64..127 at no
    extra moving-row cost, so normalization is one
    reciprocal_approx_fast [64,1024] + one tensor_mul [64,1024] per
    stream on DVE - no partition_broadcast, no slow reciprocal.
  - Scores for both q-halves land in one 2-bank [128,1024] PSUM tile:
    mask-mul is a single DVE op and exp a single ACT op per (stream,kc).
  - b1's projections ride as PE fillers inside the first two attention
    pairs; b0's out-projection rides inside b1's attention; only b1's
    out-projection is tail.
  - Batched DMAs (one per weight tensor / x batch / mask head,
    [128,1024] f16 output stores), all issued from SP.
"""

import numpy as np
import ml_dtypes

DIM = 1024
H = 16
HD = 64
B = 4
N = 1024
NCORES = 8
HPC = 4            # heads per core
BPC = 2            # batches per core
NSTACK = 2         # 2-head stacks per core
VBLK = 2 * HD      # v block width per head (64 v cols + 64 ones cols)
VROW = HPC * VBLK  # v columns per 128-token chunk

KC = DIM // 128  # 8 contraction chunks over D
TC = N // 128    # 8 token chunks
QH = N // 512    # 2 q halves (fp32 PSUM bank limit is 512)

_PROGRAM = None
LAST_RESULTS = None  # BassKernelResults from the most recent run (for test.py)


def _build_program():
    import concourse.mybir as mybir
    import concourse.tile as tile
    from concourse import bacc

    f32 = mybir.dt.float32
    f16 = mybir.dt.float16
    AF = mybir.ActivationFunctionType

    nc = bacc.Bacc(
        "TRN2",
        target_bir_lowering=False,
        debug=False,
        num_devices=NCORES,
    )

    xT = nc.dram_tensor("xT", [BPC, DIM, N], f16, kind="ExternalInput").ap()
    maskT = nc.dram_tensor("maskT", [HPC, N, N], f16, kind="ExternalInput").ap()
    wqT = nc.dram_tensor("wqT", [DIM, HPC * HD], f16, kind="ExternalInput").ap()
    wkT = nc.dram_tensor("wkT", [DIM, HPC * HD], f16, kind="ExternalInput").ap()
    wvT = nc.dram_tensor("wvT", [DIM, HPC * HD], f16, kind="ExternalInput").ap()
    woT = nc.dram_tensor("woT", [HPC * HD, DIM], f16, kind="ExternalInput").ap()
    bqd = nc.dram_tensor("bq", [NSTACK, 128, 1], f32, kind="ExternalInput").ap()
    bkd = nc.dram_tensor("bk", [NSTACK, 128, 1], f32, kind="ExternalInput").ap()
    outp = nc.dram_tensor("outp", [BPC, N, DIM], f16, kind="ExternalOutput").ap()

    with tile.TileContext(nc) as tc:
        with (
            tc.tile_pool(name="w", bufs=1) as wpool,
            tc.tile_pool(name="persist", bufs=1) as persist,
            tc.tile_pool(name="expm", bufs=3) as expm_p,
            tc.tile_pool(name="expo", bufs=3) as expo_p,
            tc.tile_pool(name="ev", bufs=2) as ev_p,
            tc.tile_pool(name="small", bufs=4) as small_p,
            tc.tile_pool(name="psB", bufs=2, space="PSUM") as psB,
            tc.tile_pool(name="psO", bufs=1, space="PSUM") as psO,
        ):
            # ---- persistent SBUF tiles -------------------------------------
            wq_t = wpool.tile([128, KC * 256], f16, tag="wq", name="wq")
            wk_t = wpool.tile([128, KC * 256], f16, tag="wk", name="wk")
            wv_t = wpool.tile([128, KC * 256], f16, tag="wv", name="wv")
            wo_t = wpool.tile([128, NSTACK * DIM], f16, tag="wo", name="wo")
            bq_t = [wpool.tile([128, 1], f32, tag=f"bq{s}", name=f"bq{s}") for s in range(NSTACK)]
            bk_t = [wpool.tile([128, 1], f32, tag=f"bk{s}", name=f"bk{s}") for s in range(NSTACK)]

            xts = {b: persist.tile([128, KC * N], f16, tag=f"x{b}", name=f"x{b}")
                   for b in range(BPC)}
            mk = {h: persist.tile([128, TC * N], f16, tag=f"mk{h}", name=f"mk{h}")
                  for h in range(HPC)}

            qT = {}
            kT = {}
            ao = {}
            for b in range(BPC):
                for s in range(NSTACK):
                    qT[(b, s)] = persist.tile([128, N], f16, tag=f"qT{b}{s}", name=f"qT{b}{s}")
                    kT[(b, s)] = persist.tile([128, N], f16, tag=f"kT{b}{s}", name=f"kT{b}{s}")
                    ao[(b, s)] = persist.tile([128, N], f16, tag=f"ao{b}{s}", name=f"ao{b}{s}")
            vt = {b: persist.tile([128, TC * VROW], f16, tag=f"v{b}", name=f"v{b}")
                  for b in range(BPC)}
            # whole-tile ones fill (on Pool; SBUF-only engine); v evacs
            # overwrite columns 0..63 of each head block, leaving columns
            # 64..127 = 1.0 (replicated-denominator columns)
            nc.gpsimd.memset(vt[0][:], 1.0)
            nc.gpsimd.memset(vt[1][:], 1.0)

            # ---- input DMAs (all on SP queue, in consumption order) --------
            def load_inputs():
                half = (KC // 2) * 128

                def chunked(dst, src, w):
                    nc.sync.dma_start(
                        dst.rearrange("p (c n) -> p c n", n=w),
                        src.rearrange("(c p) n -> p c n", p=128),
                    )

                chunked(xts[0][:, 0:KC // 2 * N], xT[0, 0:half, :], N)
                chunked(wq_t[:], wqT, 256)
                chunked(wk_t[:], wkT, 256)
                for s in range(NSTACK):
                    nc.sync.dma_start(bq_t[s][:], bqd[s])
                    nc.sync.dma_start(bk_t[s][:], bkd[s])
                chunked(xts[0][:, KC // 2 * N:], xT[0, half:, :], N)
                chunked(wv_t[:], wvT, 256)
                chunked(xts[1][:], xT[1], N)
                chunked(wo_t[:], woT, DIM)
                for h in range(HPC):
                    chunked(mk[h][:], maskT[h], N)

            # ---- projections (PSUM tiles from the shared 2-bank ring) ------
            def qk_group(b, which, s, qh):
                wt, bt, dst = ((wq_t, bq_t, qT) if which == "q"
                               else (wk_t, bk_t, kT))
                ps = psB.tile([128, N], f32, tag="big", name=f"pj{which}{b}{s}{qh}")
                for kc in range(KC):
                    nc.tensor.matmul(
                        ps[:, 0:512],
                        lhsT=wt[:, kc * 256 + s * 128:kc * 256 + (s + 1) * 128],
                        rhs=xts[b][:, kc * N + qh * 512:kc * N + (qh + 1) * 512],
                        start=(kc == 0),
                        stop=(kc == KC - 1),
                    )
                nc.scalar.activation(
                    dst[(b, s)][:, qh * 512:(qh + 1) * 512],
                    ps[:, 0:512],
                    AF.Identity,
                    bias=bt[s][:],
                    scale=1.0,
                )

            def v_group(b, tci):
                ps = psB.tile([128, N], f32, tag="big", name=f"pjv{b}{tci}")
                for kc in range(KC):
                    nc.tensor.matmul(
                        ps[:, 0:HPC * HD],
                        lhsT=xts[b][:, kc * N + tci * 128:kc * N + (tci + 1) * 128],
                        rhs=wv_t[:, kc * 256:(kc + 1) * 256],
                        start=(kc == 0),
                        stop=(kc == KC - 1),
                    )
                dst = vt[b][:, tci * VROW:(tci + 1) * VROW].rearrange(
                    "p (h e) -> p h e", e=VBLK
                )[:, :, 0:HD]
                nc.scalar.activation(
                    dst, ps[:, 0:HPC * HD].rearrange("p (h e) -> p h e", e=HD),
                    AF.Copy,
                )

            def proj_groups(b):
                out = []
                for which in ("q", "k"):
                    for s in range(NSTACK):
                        for qh in range(QH):
                            out.append(lambda b=b, w=which, s=s, qh=qh:
                                       qk_group(b, w, s, qh))
                for tci in range(TC):
                    out.append(lambda b=b, tci=tci: v_group(b, tci))
                return out

            # ---- attention stream machinery --------------------------------
            def sc_mm(st, kc):
                """both q-half score matmuls into one 2-bank PSUM tile"""
                h, b = st
                s, hh = h // 2, h % 2
                sc = psB.tile([128, N], f32, tag="big", name=f"sc{h}{b}{kc}")
                for qh in range(QH):
                    nc.tensor.matmul(
                        sc[:, qh * 512:(qh + 1) * 512],
                        lhsT=kT[(b, s)][hh * HD:(hh + 1) * HD,
                                        kc * 128:(kc + 1) * 128],
                        rhs=qT[(b, s)][hh * HD:(hh + 1) * HD,
                                       qh * 512:(qh + 1) * 512],
                        start=True,
                        stop=True,
                    )
                return sc

            def mul_exp(st, kc, sc):
                h, b = st
                em = expm_p.tile([128, N], f32, tag="expm", name=f"em{h}{b}{kc}")
                nc.vector.tensor_mul(
                    em[:], sc[:], mk[h][:, kc * N:(kc + 1) * N])
                eo = expo_p.tile([128, N], f16, tag="expo", name=f"eo{h}{b}{kc}")
                nc.scalar.activation(eo[:], em[:], AF.Exp)
                return eo

            def av_mm(st, kc, eo, op):
                h, b = st
                vblk = vt[b][:, kc * VROW + h * VBLK:kc * VROW + (h + 1) * VBLK]
                for qh in range(QH):
                    nc.tensor.matmul(
                        op[:, qh * 512:(qh + 1) * 512],
                        lhsT=vblk,
                        rhs=eo[:, qh * 512:(qh + 1) * 512],
                        start=(kc == 0),
                        stop=(kc == TC - 1),
                    )

            def stream_finish(st, op):
                """reciprocal of the replicated denominator rows + normalize
                into ao (both [64,1024] DVE ops); frees op for the next
                stream on this psO slot."""
                h, b = st
                s, hh = h // 2, h % 2
                bcs = small_p.tile([HD, N], f32, tag="bcs", name=f"bcs{h}{b}")
                nc.vector.reciprocal(bcs[:], op[HD:2 * HD, :])
                nc.vector.tensor_mul(
                    ao[(b, s)][hh * HD:(hh + 1) * HD, :],
                    op[0:HD, :],
                    bcs[:],
                )

            # ---- out-projection (merged dh; one evac + one store) ----------
            def outproj_group(b, tci):
                po = psB.tile([128, N], f32, tag="big", name=f"po{b}{tci}")
                for dh in range(QH):
                    for s in range(NSTACK):
                        nc.tensor.matmul(
                            po[:, dh * 512:(dh + 1) * 512],
                            lhsT=ao[(b, s)][:, tci * 128:(tci + 1) * 128],
                            rhs=wo_t[:, s * DIM + dh * 512:s * DIM + (dh + 1) * 512],
                            start=(s == 0),
                            stop=(s == NSTACK - 1),
                        )
                ot = ev_p.tile([128, N], f16, tag="ot", name=f"ot{b}{tci}")
                nc.scalar.activation(ot[:], po[:], AF.Copy)
                nc.sync.dma_start(outp[b, tci * 128:(tci + 1) * 128, :], ot[:])

            # ================= emission =====================================
            load_inputs()

            for g in proj_groups(0):
                g()

            # stream pairs, batch-major; fillers: b1 projections ride in
            # pairs 0-1, b0 out-projection rides in pairs 2-3
            pairs = [((0, 0), (1, 0)), ((2, 0), (3, 0)),
                     ((0, 1), (1, 1)), ((2, 1), (3, 1))]
            fill = {0: proj_groups(1)[0:8], 1: proj_groups(1)[8:16]}

            opt = {}
            for pi, (A, Bst) in enumerate(pairs):
                for slot, st in ((0, A), (1, Bst)):
                    opt[slot] = psO.tile([128, N], f32, tag=f"op{slot}",
                                         name=f"op{st[0]}{st[1]}")
                prev = None
                for kc in range(TC):
                    fl = fill.get(pi)
                    if fl:
                        fl.pop(0)()
                    scA = sc_mm(A, kc)
                    scB = sc_mm(Bst, kc)
                    if prev is not None:
                        av_mm(A, prev[0], prev[1], opt[0])
                        av_mm(Bst, prev[0], prev[2], opt[1])
                    eoA = mul_exp(A, kc, scA)
                    eoB = mul_exp(Bst, kc, scB)
                    prev = (kc, eoA, eoB)
                av_mm(A, prev[0], prev[1], opt[0])
                av_mm(Bst, prev[0], prev[2], opt[1])
                stream_finish(A, opt[0])
                stream_finish(Bst, opt[1])
                if pi == 1:
                    fill[2] = [lambda tci=tci: outproj_group(0, tci)
                               for tci in range(0, TC, 2)]
                    fill[3] = [lambda tci=tci: outproj_group(0, tci)
                               for tci in range(1, TC, 2)]
            for tci in range(TC):
                outproj_group(1, tci)

    nc.compile()
    return nc


def _get_program():
    global _PROGRAM
    if _PROGRAM is None:
        _PROGRAM = _build_program()
    return _PROGRAM


def kernel(x, decaymask, wq, bq, wk, bk, wv, bv, wo, bo):
    from concourse.bass_utils import run_bass_kernel_spmd

    global LAST_RESULTS

    x = np.ascontiguousarray(np.asarray(x, dtype=np.float32))
    decaymask = np.ascontiguousarray(np.asarray(decaymask, dtype=np.float32))
    wq = np.asarray(wq, dtype=np.float32)
    bq = np.asarray(bq, dtype=np.float32)
    wk = np.asarray(wk, dtype=np.float32)
    bk = np.asarray(bk, dtype=np.float32)
    wv = np.asarray(wv, dtype=np.float32)
    bv = np.asarray(bv, dtype=np.float32)
    wo = np.asarray(wo, dtype=np.float32)
    bo = np.asarray(bo, dtype=np.float32)

    nc = _get_program()

    in_maps = []
    for c in range(NCORES):
        g, p = c // 2, c % 2
        rows = slice(g * HPC * HD, (g + 1) * HPC * HD)
        xT_c = np.ascontiguousarray(
            x[p * BPC:(p + 1) * BPC].transpose(0, 2, 1)
        ).astype(np.float16)  # [BPC, D, N]
        maskT_c = np.ascontiguousarray(
            decaymask[g * HPC:(g + 1) * HPC].transpose(0, 2, 1)
        ).astype(np.float16)  # [HPC, k, q]
        # fold 1/sqrt(HD) = 1/8 (exact) into wq/bq
        wqT_c = (np.ascontiguousarray(wq[rows, :].T) * np.float32(0.125)).astype(np.float16)
        wkT_c = np.ascontiguousarray(wk[rows, :].T).astype(np.float16)
        wvT_c = np.ascontiguousarray(wv[rows, :].T).astype(np.float16)
        woT_c = np.ascontiguousarray(wo[:, rows].T).astype(np.float16)
        bq_c = (bq[rows] * np.float32(0.125)).reshape(NSTACK, 128, 1)
        bk_c = bk[rows].reshape(NSTACK, 128, 1).copy()
        in_maps.append({
            "xT": xT_c,
            "maskT": maskT_c,
            "wqT": wqT_c,
            "wkT": wkT_c,
            "wvT": wvT_c,
            "woT": woT_c,
            "bq": np.ascontiguousarray(bq_c),
            "bk": bk_c,
        })

    res = run_bass_kernel_spmd(nc, in_maps, list(range(NCORES)))
    LAST_RESULTS = res

    out = np.zeros((B, N, DIM), dtype=np.float32)
    for c in range(NCORES):
        g, p = c // 2, c % 2
        out[p * BPC:(p + 1) * BPC] += res.results[c]["outp"].astype(np.float32)
    out += (bo + bv @ wo.T)[None, None, :]
    return out


# revision 8
# speedup vs baseline: 1.4122x; 1.1717x over previous
"""DecayMaskedMultiHeadAttention on 8 trn2 NeuronCores (Bass/Tile SPMD).

Model: B=4, N=1024, DIM=1024, 16 heads x head_dim 64.
  q/k/v = x @ W.T + b ; scores = (q_h k_h^T)/8 * decaymask_h ;
  out = softmax(scores) v_h ; y = concat_h(out) @ wo.T + bo

Sharding (8 cores): 4 head-groups x 2 batch-groups.
  core c: head group g = c // 2 (heads 4g..4g+3), batch group p = c % 2
  (batches 2p, 2p+1). Each core computes a partial y (f16) for its 2
  batches; host sums the 4 partials per batch group and adds the
  closed-form bias terms (bo + bv @ wo.T; attention rows sum to 1 so bv
  passes through).

Schedule notes (TRN2: Pool/GPSIMD cannot touch PSUM, so all PSUM-side
element-wise work lives on DVE + ACT):
  - Attention runs as TWO concurrent (head, batch) streams interleaved
    kc-by-kc with the attn@v matmuls software-pipelined one kc behind
    the scores, so the PE never waits on a single
    score->mask-mul->exp->attn@v chain (PE duty keeps the clock p-state
    at 2.4 GHz).
  - v tiles carry 64 ones-columns per head: the attn@v matmul emits the
    softmax denominator already replicated on partitions 64..127 at no
    extra moving-row cost, so normalization is one
    reciprocal_approx_fast [64,1024] + one tensor_mul [64,1024] per
    stream on DVE - no partition_broadcast, no slow reciprocal.
  - Scores for both q-halves land in one 2-bank [128,1024] PSUM tile:
    mask-mul is a single DVE op and exp a single ACT op per (stream,kc).
  - b1's projections ride as PE fillers inside the first two attention
    pairs; b0's out-projection rides inside b1's attention; only b1's
    out-projection is tail.
  - Batched DMAs (one per weight tensor / x batch / mask head,
    [128,1024] f16 output stores), all issued from SP.
"""

import numpy as np
import ml_dtypes

DIM = 1024
H = 16
HD = 64
B = 4
N = 1024
NCORES = 8
HPC = 4            # heads per core
BPC = 2            # batches per core
NSTACK = 2         # 2-head stacks per core
VBLK = 2 * HD      # v block width per head (64 v cols + 64 ones cols)
VROW = HPC * VBLK  # v columns per 128-token chunk

KC = DIM // 128  # 8 contraction chunks over D
TC = N // 128    # 8 token chunks
QH = N // 512    # 2 q halves (fp32 PSUM bank limit is 512)

_PROGRAM = None
LAST_RESULTS = None  # BassKernelResults from the most recent run (for test.py)


def _build_program():
    import concourse.mybir as mybir
    import concourse.tile as tile
    from concourse import bacc

    f32 = mybir.dt.float32
    f16 = mybir.dt.float16
    AF = mybir.ActivationFunctionType

    nc = bacc.Bacc(
        "TRN2",
        target_bir_lowering=False,
        debug=False,
        num_devices=NCORES,
    )

    xT = nc.dram_tensor("xT", [BPC, DIM, N], f16, kind="ExternalInput").ap()
    maskT = nc.dram_tensor("maskT", [HPC, N, N], f16, kind="ExternalInput").ap()
    wqT = nc.dram_tensor("wqT", [DIM, HPC * HD], f16, kind="ExternalInput").ap()
    wkT = nc.dram_tensor("wkT", [DIM, HPC * HD], f16, kind="ExternalInput").ap()
    wvT = nc.dram_tensor("wvT", [DIM, HPC * HD], f16, kind="ExternalInput").ap()
    woT = nc.dram_tensor("woT", [HPC * HD, DIM], f16, kind="ExternalInput").ap()
    bqd = nc.dram_tensor("bq", [NSTACK, 128, 1], f32, kind="ExternalInput").ap()
    bkd = nc.dram_tensor("bk", [NSTACK, 128, 1], f32, kind="ExternalInput").ap()
    outp = nc.dram_tensor("outp", [BPC, N, DIM], f16, kind="ExternalOutput").ap()

    with tile.TileContext(nc) as tc:
        with (
            tc.tile_pool(name="w", bufs=1) as wpool,
            tc.tile_pool(name="persist", bufs=1) as persist,
            tc.tile_pool(name="expm", bufs=3) as expm_p,
            tc.tile_pool(name="expo", bufs=3) as expo_p,
            tc.tile_pool(name="ev", bufs=2) as ev_p,
            tc.tile_pool(name="small", bufs=4) as small_p,
            tc.tile_pool(name="psB", bufs=2, space="PSUM") as psB,
            tc.tile_pool(name="psO", bufs=1, space="PSUM") as psO,
        ):
            # ---- persistent SBUF tiles -------------------------------------
            wq_t = wpool.tile([128, KC * 256], f16, tag="wq", name="wq")
            wk_t = wpool.tile([128, KC * 256], f16, tag="wk", name="wk")
            wv_t = wpool.tile([128, KC * 256], f16, tag="wv", name="wv")
            wo_t = wpool.tile([128, NSTACK * DIM], f16, tag="wo", name="wo")
            bq_t = [wpool.tile([128, 1], f32, tag=f"bq{s}", name=f"bq{s}") for s in range(NSTACK)]
            bk_t = [wpool.tile([128, 1], f32, tag=f"bk{s}", name=f"bk{s}") for s in range(NSTACK)]

            xts = {b: persist.tile([128, KC * N], f16, tag=f"x{b}", name=f"x{b}")
                   for b in range(BPC)}
            mk = {h: persist.tile([128, TC * N], f16, tag=f"mk{h}", name=f"mk{h}")
                  for h in range(HPC)}

            qT = {}
            kT = {}
            ao = {}
            for b in range(BPC):
                for s in range(NSTACK):
                    qT[(b, s)] = persist.tile([128, N], f16, tag=f"qT{b}{s}", name=f"qT{b}{s}")
                    kT[(b, s)] = persist.tile([128, N], f16, tag=f"kT{b}{s}", name=f"kT{b}{s}")
                    ao[(b, s)] = persist.tile([128, N], f16, tag=f"ao{b}{s}", name=f"ao{b}{s}")
            vt = {b: persist.tile([128, TC * VROW], f16, tag=f"v{b}", name=f"v{b}")
                  for b in range(BPC)}
            # whole-tile ones fill (on Pool; SBUF-only engine); v evacs
            # overwrite columns 0..63 of each head block, leaving columns
            # 64..127 = 1.0 (replicated-denominator columns)
            nc.gpsimd.memset(vt[0][:], 1.0)
            nc.gpsimd.memset(vt[1][:], 1.0)

            # ---- input DMAs (all on SP queue, in consumption order) --------
            def load_inputs():
                half = (KC // 2) * 128

                def chunked(dst, src, w):
                    nc.sync.dma_start(
                        dst.rearrange("p (c n) -> p c n", n=w),
                        src.rearrange("(c p) n -> p c n", p=128),
                    )

                chunked(xts[0][:, 0:KC // 2 * N], xT[0, 0:half, :], N)
                chunked(wq_t[:], wqT, 256)
                chunked(wk_t[:], wkT, 256)
                for s in range(NSTACK):
                    nc.sync.dma_start(bq_t[s][:], bqd[s])
                    nc.sync.dma_start(bk_t[s][:], bkd[s])
                chunked(xts[0][:, KC // 2 * N:], xT[0, half:, :], N)
                chunked(wv_t[:], wvT, 256)
                chunked(xts[1][:], xT[1], N)
                chunked(wo_t[:], woT, DIM)
                for h in range(HPC):
                    chunked(mk[h][:], maskT[h], N)

            # ---- projections (PSUM tiles from the shared 2-bank ring) ------
            def qk_group(b, which, s, qh):
                wt, bt, dst = ((wq_t, bq_t, qT) if which == "q"
                               else (wk_t, bk_t, kT))
                ps = psB.tile([128, N], f32, tag="big", name=f"pj{which}{b}{s}{qh}")
                for kc in range(KC):
                    nc.tensor.matmul(
                        ps[:, 0:512],
                        lhsT=wt[:, kc * 256 + s * 128:kc * 256 + (s + 1) * 128],
                        rhs=xts[b][:, kc * N + qh * 512:kc * N + (qh + 1) * 512],
                        start=(kc == 0),
                        stop=(kc == KC - 1),
                    )
                nc.scalar.activation(
                    dst[(b, s)][:, qh * 512:(qh + 1) * 512],
                    ps[:, 0:512],
                    AF.Identity,
                    bias=bt[s][:],
                    scale=1.0,
                )

            def v_group(b, tci):
                ps = psB.tile([128, N], f32, tag="big", name=f"pjv{b}{tci}")
                for kc in range(KC):
                    nc.tensor.matmul(
                        ps[:, 0:HPC * HD],
                        lhsT=xts[b][:, kc * N + tci * 128:kc * N + (tci + 1) * 128],
                        rhs=wv_t[:, kc * 256:(kc + 1) * 256],
                        start=(kc == 0),
                        stop=(kc == KC - 1),
                    )
                dst = vt[b][:, tci * VROW:(tci + 1) * VROW].rearrange(
                    "p (h e) -> p h e", e=VBLK
                )[:, :, 0:HD]
                nc.scalar.activation(
                    dst, ps[:, 0:HPC * HD].rearrange("p (h e) -> p h e", e=HD),
                    AF.Copy,
                )

            def proj_groups(b):
                out = []
                for which in ("q", "k"):
                    for s in range(NSTACK):
                        for qh in range(QH):
                            out.append(lambda b=b, w=which, s=s, qh=qh:
                                       qk_group(b, w, s, qh))
                for tci in range(TC):
                    out.append(lambda b=b, tci=tci: v_group(b, tci))
                return out

            # ---- attention stream machinery --------------------------------
            def sc_mm(st, kc):
                """both q-half score matmuls into one 2-bank PSUM tile"""
                h, b = st
                s, hh = h // 2, h % 2
                sc = psB.tile([128, N], f32, tag="big", name=f"sc{h}{b}{kc}")
                for qh in range(QH):
                    nc.tensor.matmul(
                        sc[:, qh * 512:(qh + 1) * 512],
                        lhsT=kT[(b, s)][hh * HD:(hh + 1) * HD,
                                        kc * 128:(kc + 1) * 128],
                        rhs=qT[(b, s)][hh * HD:(hh + 1) * HD,
                                       qh * 512:(qh + 1) * 512],
                        start=True,
                        stop=True,
                    )
                return sc

            def mul_exp(st, kc, sc):
                h, b = st
                em = expm_p.tile([128, N], f32, tag="expm", name=f"em{h}{b}{kc}")
                nc.vector.tensor_mul(
                    em[:], sc[:], mk[h][:, kc * N:(kc + 1) * N])
                eo = expo_p.tile([128, N], f16, tag="expo", name=f"eo{h}{b}{kc}")
                nc.scalar.activation(eo[:], em[:], AF.Exp)
                return eo

            def av_mm(st, kc, eo, op):
                h, b = st
                vblk = vt[b][:, kc * VROW + h * VBLK:kc * VROW + (h + 1) * VBLK]
                for qh in range(QH):
                    nc.tensor.matmul(
                        op[:, qh * 512:(qh + 1) * 512],
                        lhsT=vblk,
                        rhs=eo[:, qh * 512:(qh + 1) * 512],
                        start=(kc == 0),
                        stop=(kc == TC - 1),
                    )

            def stream_finish(st, op):
                """reciprocal of the replicated denominator rows + normalize
                into ao (both [64,1024] DVE ops); frees op for the next
                stream on this psO slot."""
                h, b = st
                s, hh = h // 2, h % 2
                dn = small_p.tile([HD, N], f32, tag="dn", name=f"dn{h}{b}")
                nc.scalar.activation(dn[:], op[HD:2 * HD, :], AF.Copy)
                bcs = small_p.tile([HD, N], f32, tag="bcs", name=f"bcs{h}{b}")
                nc.vector.reciprocal_approx_fast(bcs[:], dn[:])
                nc.vector.tensor_mul(
                    ao[(b, s)][hh * HD:(hh + 1) * HD, :],
                    op[0:HD, :],
                    bcs[:],
                )

            # ---- out-projection (merged dh; one evac + one store) ----------
            def outproj_group(b, tci):
                po = psB.tile([128, N], f32, tag="big", name=f"po{b}{tci}")
                for dh in range(QH):
                    for s in range(NSTACK):
                        nc.tensor.matmul(
                            po[:, dh * 512:(dh + 1) * 512],
                            lhsT=ao[(b, s)][:, tci * 128:(tci + 1) * 128],
                            rhs=wo_t[:, s * DIM + dh * 512:s * DIM + (dh + 1) * 512],
                            start=(s == 0),
                            stop=(s == NSTACK - 1),
                        )
                ot = ev_p.tile([128, N], f16, tag="ot", name=f"ot{b}{tci}")
                nc.scalar.activation(ot[:], po[:], AF.Copy)
                nc.sync.dma_start(outp[b, tci * 128:(tci + 1) * 128, :], ot[:])

            # ================= emission =====================================
            load_inputs()

            for g in proj_groups(0):
                g()

            # stream pairs, batch-major; fillers: b1 projections ride in
            # pairs 0-1, b0 out-projection rides in pairs 2-3
            pairs = [((0, 0), (1, 0)), ((2, 0), (3, 0)),
                     ((0, 1), (1, 1)), ((2, 1), (3, 1))]
            fill = {0: proj_groups(1)[0:8], 1: proj_groups(1)[8:16]}

            opt = {}
            for pi, (A, Bst) in enumerate(pairs):
                for slot, st in ((0, A), (1, Bst)):
                    opt[slot] = psO.tile([128, N], f32, tag=f"op{slot}",
                                         name=f"op{st[0]}{st[1]}")
                prev = None
                for kc in range(TC):
                    fl = fill.get(pi)
                    if fl:
                        fl.pop(0)()
                    scA = sc_mm(A, kc)
                    scB = sc_mm(Bst, kc)
                    if prev is not None:
                        av_mm(A, prev[0], prev[1], opt[0])
                        av_mm(Bst, prev[0], prev[2], opt[1])
                    eoA = mul_exp(A, kc, scA)
                    eoB = mul_exp(Bst, kc, scB)
                    prev = (kc, eoA, eoB)
                av_mm(A, prev[0], prev[1], opt[0])
                av_mm(Bst, prev[0], prev[2], opt[1])
                stream_finish(A, opt[0])
                stream_finish(Bst, opt[1])
                if pi == 1:
                    fill[2] = [lambda tci=tci: outproj_group(0, tci)
                               for tci in range(0, TC, 2)]
                    fill[3] = [lambda tci=tci: outproj_group(0, tci)
                               for tci in range(1, TC, 2)]
            for tci in range(TC):
                outproj_group(1, tci)

    nc.compile()
    return nc


def _get_program():
    global _PROGRAM
    if _PROGRAM is None:
        _PROGRAM = _build_program()
    return _PROGRAM


def kernel(x, decaymask, wq, bq, wk, bk, wv, bv, wo, bo):
    from concourse.bass_utils import run_bass_kernel_spmd

    global LAST_RESULTS

    x = np.ascontiguousarray(np.asarray(x, dtype=np.float32))
    decaymask = np.ascontiguousarray(np.asarray(decaymask, dtype=np.float32))
    wq = np.asarray(wq, dtype=np.float32)
    bq = np.asarray(bq, dtype=np.float32)
    wk = np.asarray(wk, dtype=np.float32)
    bk = np.asarray(bk, dtype=np.float32)
    wv = np.asarray(wv, dtype=np.float32)
    bv = np.asarray(bv, dtype=np.float32)
    wo = np.asarray(wo, dtype=np.float32)
    bo = np.asarray(bo, dtype=np.float32)

    nc = _get_program()

    in_maps = []
    for c in range(NCORES):
        g, p = c // 2, c % 2
        rows = slice(g * HPC * HD, (g + 1) * HPC * HD)
        xT_c = np.ascontiguousarray(
            x[p * BPC:(p + 1) * BPC].transpose(0, 2, 1)
        ).astype(np.float16)  # [BPC, D, N]
        maskT_c = np.ascontiguousarray(
            decaymask[g * HPC:(g + 1) * HPC].transpose(0, 2, 1)
        ).astype(np.float16)  # [HPC, k, q]
        # fold 1/sqrt(HD) = 1/8 (exact) into wq/bq
        wqT_c = (np.ascontiguousarray(wq[rows, :].T) * np.float32(0.125)).astype(np.float16)
        wkT_c = np.ascontiguousarray(wk[rows, :].T).astype(np.float16)
        wvT_c = np.ascontiguousarray(wv[rows, :].T).astype(np.float16)
        woT_c = np.ascontiguousarray(wo[:, rows].T).astype(np.float16)
        bq_c = (bq[rows] * np.float32(0.125)).reshape(NSTACK, 128, 1)
        bk_c = bk[rows].reshape(NSTACK, 128, 1).copy()
        in_maps.append({
            "xT": xT_c,
            "maskT": maskT_c,
            "wqT": wqT_c,
            "wkT": wkT_c,
            "wvT": wvT_c,
            "woT": woT_c,
            "bq": np.ascontiguousarray(bq_c),
            "bk": bk_c,
        })

    res = run_bass_kernel_spmd(nc, in_maps, list(range(NCORES)))
    LAST_RESULTS = res

    out = np.zeros((B, N, DIM), dtype=np.float32)
    for c in range(NCORES):
        g, p = c // 2, c % 2
        out[p * BPC:(p + 1) * BPC] += res.results[c]["outp"].astype(np.float32)
    out += (bo + bv @ wo.T)[None, None, :]
    return out
